# revision 23
# baseline (speedup 1.0000x reference)
"""Trainium2 Bass kernel for nn_CrossAttention (B=8, N1=64, N2=4096, C=768, H=12).

Strategy: data-parallel over batch across 8 NeuronCores (one item per core,
no collectives). All activations kept transposed (channels on partitions,
tokens on the free dim) so every matmul contracts over SBUF partitions.

Key algebraic restructurings (exploiting that the module's combine with v is
ELEMENTWISE, attn_t * v, not attn @ v):

  1. scores_h = q_h @ k_h^T = (q_h @ W_k_h) @ yT = A_h @ yT.  A = qT^T @ W_k
     is a tiny [768,768] precompute; scores then contract over the full
     K=128 partitions with the SAME moving operand (yT chunks) as the
     v-projection — k is never materialized.
  2. softmax normalization is deferred: U_h = exp(s_h) * vT_h is accumulated
     unnormalized; row-sums S come free via ACT's fused accum_out; 1/S is
     folded into the projection weights (O(C^2), not O(C*N2)).

fp8 acceleration (DoubleRow perf mode, 2 contraction tiles per pass):
  - v-projection runs as an error-compensated fp8 "double-double":
    host ships hi/lo e4m3 splits of both W_v (x64) and yT (x16), and the
    kernel accumulates Whi.yhi + Wlo.yhi + Whi.ylo (9 DoubleRow passes vs
    12 bf16 passes worth of PE rows) -- quantization error ~eps^2.
  - scores run in plain fp8: A is quantized on device (x128) against the
    hi yT split. Score errors are absolute-small and further damped by
    softmax's exp; end-to-end rel err stays ~1.2e-2 (gate 2e-2).
  - the output projection stays bf16.

Matmuls accumulate in f32 PSUM; softmax statistics in f32.
DMAs are batched; the input stream is split across the SP and ACT queues so
the first PE work (qT projection) and the chunk-0 v-projection hi terms can
start as early as possible.
"""

import numpy as np
import ml_dtypes

import concourse.bass as bass
import concourse.mybir as mybir
import concourse.tile as tile
from concourse import bacc
from concourse.bass_utils import run_bass_kernel_spmd

BF16 = mybir.dt.bfloat16
FP8 = mybir.dt.float8e4
F32 = mybir.dt.float32
DR = mybir.MatmulPerfMode.DoubleRow

B, N1, N2, C, H = 8, 64, 4096, 768, 12
HD = C // H              # 64
SCALE = HD ** -0.5       # 1/8
CT = C // 128            # 6 partition tiles of channels
CT2 = 2 * CT             # 12 fp8 k-tiles (hi | lo)
CHUNK = 1024             # tokens per outproj block
NCH = N2 // CHUNK        # 4 outproj blocks
# scores/vproj stream chunks: last chunk halved so the scores->outproj
# boundary only waits for 6 exps instead of 12
CHUNKS = [(0, 1024), (1024, 1024), (2048, 1024), (3072, 512), (3584, 512)]
SLOT0 = [0, 2, 4, 6, 7]  # S_parts slot base per chunk (one per 512 tokens)
PAIRS = CT               # 6 head pairs (2 heads per 128-partition tile)

# quantization scales (powers of two; folded back out exactly)
SY = 16.0                # yT shipped as e4m3(16*y) + residual
SWV = 64.0               # W_v^T shipped as e4m3(64*Wv) + residual
SA = 128.0               # A quantized on device as e4m3(128*A)
VDRAIN = 1.0 / (SY * SWV)        # 2^-10: v psum -> true v
# qbd already folds SCALE=1/8, so psum_s = (128*A)@(16*y) = 2048*s
EXPSCALE = 1.0 / (SA * SY)       # 2^-11
ALPHA = 8.0                      # E' = 8*E via exp bias ln(8): fp8 range for U
SWP = 2.0 ** 18                  # W_proj^T shipped pre-scaled by 2^18
SD = 1.0 / SWP                   # out-proj psum drain scale

BUFS_YT = 2
BUFS_VT = 2
BUFS_E = 6
BUFS_PSKV = 3
BUFS_PSS = 1

_CACHE = {}


def _build():
    nc = bacc.Bacc("TRN2", target_bir_lowering=False, debug=False)

    xT_d = nc.dram_tensor("xT", [C, N1], BF16, kind="ExternalInput")
    # yq: rows 0..C-1 = e4m3(16*yT), rows C..2C-1 = residual
    yq_d = nc.dram_tensor("yq", [2 * C, N2], FP8, kind="ExternalInput")
    # wq: W_q^T layout [c_in, c_out]
    wq_d = nc.dram_tensor("wq", [C, C], BF16, kind="ExternalInput")
    # wk: natural layout [c_out, c_in] = W_qkv[C:2C, :]
    wk_d = nc.dram_tensor("wk", [C, C], BF16, kind="ExternalInput")
    # wvq: [Wv_hi | Wv_lo] stacked on rows; each [c_in, c_out] = e4m3(64*Wv^T)
    wvq_d = nc.dram_tensor("wvq", [2 * C, C], FP8, kind="ExternalInput")
    wprojT_d = nc.dram_tensor("wprojT", [C, C], BF16, kind="ExternalInput")
    bproj_d = nc.dram_tensor("bproj", [C, 1], F32, kind="ExternalInput")
    outT_d = nc.dram_tensor("outT", [C, N2], BF16, kind="ExternalOutput")

    def t6(ap):  # [768, X] dram view -> [128, 6, X] partition-tiled view
        return ap.rearrange("(t p) c -> p t c", p=128)

    with tile.TileContext(nc) as tc:
        with (
            tc.tile_pool(name="persist", bufs=1) as pp,
            tc.tile_pool(name="work", bufs=2) as wp,
            tc.tile_pool(name="psum", bufs=2, space=bass.MemorySpace.PSUM) as psp,
        ):
            # ---- persistent tiles (partition-tiled: [:, kk, :] = rows of 128)
            wq_sb = pp.tile([128, CT, C], BF16, name="wq", tag="wq")
            wv_sb = pp.tile([128, CT2, C], FP8, name="wv", tag="wv")
            wk_sb = pp.tile([128, CT, C], BF16, name="wk", tag="wk")
            wp_sb = pp.tile([128, CT, C], BF16, name="wpr", tag="wpr")
            wps_sb = pp.tile([128, CT, C], BF16, name="wps", tag="wps")
            A_sb = pp.tile([128, CT, C], FP8, name="A", tag="A")
            xT_sb = pp.tile([128, CT, N1], BF16, name="xTs", tag="xTs")
            bias_sb = pp.tile([128, CT, 1], F32, name="biass", tag="biass")
            # block-diagonal q: qbd[0:64, g, 0:64] = qT head 2g,
            # qbd[64:128, g, 64:128] = qT head 2g+1, zeros elsewhere.
            # Lets A-prep contract K=128 in one clean full-array matmul.
            qbd = pp.tile([128, CT, 128], BF16, name="qbd", tag="qbd")
            U8 = pp.tile([128, CT2, N2], FP8, name="U8", tag="U8")
            wps8 = pp.tile([128, CT2, C], FP8, name="wps8", tag="wps8")
            S_parts = [pp.tile([128, 2 * NCH], F32, name=f"Sp{g}", tag=f"Sp{g}")
                       for g in range(PAIRS)]
            zbias = pp.tile([128, 1], F32, name="zbias", tag="zbias")
            lbias = pp.tile([128, 1], F32, name="lbias", tag="lbias")
            nc.gpsimd.memset(zbias[:], 0.0)
            nc.gpsimd.memset(lbias[:], float(np.log(ALPHA)))
            nc.gpsimd.memset(qbd[:], 0.0)

            # ---- batched weight/input DMAs ----------------------------------
            # ALL input transfers ride the sync (SP) queue: transfers
            # serialize on the shared DMA engines, so a single queue pins the
            # global order to the PE program's consumption order. Output
            # stores use the scalar queue (disjoint in time).
            nc.sync.dma_start(wv_sb[:, :CT, :], t6(wvq_d[:C, :]))

            def chunk_dma(ci):
                t0, w = CHUNKS[ci]
                yq_c = wp.tile([128, CT2, w], FP8, name="yqc", tag="yqc",
                               bufs=BUFS_YT)
                nc.sync.dma_start(
                    yq_c[:],
                    yq_d[:, t0:t0 + w].rearrange("(t p) c -> p t c", p=128))
                return yq_c

            # chunk 0 in hi / wv_lo / lo arrival order = vproj0 pass order.
            yq_next = wp.tile([128, CT2, CHUNK], FP8, name="yqc", tag="yqc",
                              bufs=BUFS_YT)
            nc.sync.dma_start(yq_next[:, :CT, :], t6(yq_d[:C, :CHUNK]))
            nc.sync.dma_start(wv_sb[:, CT:, :], t6(wvq_d[C:, :]))
            nc.sync.dma_start(yq_next[:, CT:, :], t6(yq_d[C:, :CHUNK]))

            nc.sync.dma_start(xT_sb[:], t6(xT_d[:, :]))
            nc.sync.dma_start(wq_sb[:], t6(wq_d[:, :]))
            nc.sync.dma_start(wk_sb[:], t6(wk_d[:, :]))

            def vproj_m(pskv, yq_c, m, terms, nhf=2):
                """Accumulate DoubleRow passes for out-tile m into pskv.

                terms: list of (lhs_base, rhs_base) k-tile base offsets:
                  (0, 0) = Whi.yhi, (CT, 0) = Wlo.yhi, (0, CT) = Whi.ylo.
                start on the first pass of the first term; stop on the last.
                """
                npass = 0
                total = 3 * len(terms)
                for (lb, rb) in terms:
                    for j in range(3):
                        for hf in range(nhf):
                            nc.tensor.matmul(
                                pskv[:, 512 * hf:512 * (hf + 1)],
                                wv_sb[:, lb + 2 * j:lb + 2 * j + 2,
                                      128 * m:128 * (m + 1)],
                                yq_c[:, rb + 2 * j:rb + 2 * j + 2,
                                     512 * hf:512 * (hf + 1)],
                                start=(npass == 0), stop=(npass == total - 1),
                                perf_mode=DR,
                            )
                        npass += 1

            def vdrain(vT_c, pskv, m, dve_only=False):
                if not dve_only and m % 2 == 0:
                    nc.scalar.activation(vT_c[m][:], pskv[:],
                                         mybir.ActivationFunctionType.Copy,
                                         bias=0.0, scale=VDRAIN)
                else:
                    nc.vector.tensor_scalar_mul(vT_c[m][:], pskv[:], VDRAIN)

            def vproj(ci, yq_c):
                w = CHUNKS[ci][1]
                vT_c = [wp.tile([128, w], BF16, name=f"vTc{m}", tag=f"vTc{m}",
                                bufs=BUFS_VT) for m in range(CT)]
                for m in range(CT):
                    pskv = psp.tile([128, w], F32, name="pskv", tag="pskv",
                                    bufs=BUFS_PSKV)
                    vproj_m(pskv, yq_c, m,
                            [(0, 0), (CT, 0), (0, CT)], nhf=w // 512)
                    vdrain(vT_c, pskv, m)
                return vT_c

            def vproj0(c, yq_c):
                """Chunk-0 variant ordered around DMA arrival: hi terms for
                m0-m2 run before y_lo lands; their lo-y passes close after."""
                vT_c = [wp.tile([128, CHUNK], BF16, name=f"vTc{m}", tag=f"vTc{m}",
                                bufs=BUFS_VT) for m in range(CT)]
                ps = [psp.tile([128, CHUNK], F32, name="pskv", tag="pskv",
                               bufs=BUFS_PSKV) for _ in range(3)]
                for (lb, rb) in [(0, 0), (CT, 0)]:
                    for m in range(3):
                        for j in range(3):
                            for hf in range(2):
                                nc.tensor.matmul(
                                    ps[m][:, 512 * hf:512 * (hf + 1)],
                                    wv_sb[:, lb + 2 * j:lb + 2 * j + 2,
                                          128 * m:128 * (m + 1)],
                                    yq_c[:, rb + 2 * j:rb + 2 * j + 2,
                                         512 * hf:512 * (hf + 1)],
                                    start=(lb == 0 and j == 0), stop=False,
                                    perf_mode=DR,
                                )
                for m in range(3):
                    for j in range(3):
                        for hf in range(2):
                            nc.tensor.matmul(
                                ps[m][:, 512 * hf:512 * (hf + 1)],
                                wv_sb[:, 2 * j:2 * j + 2, 128 * m:128 * (m + 1)],
                                yq_c[:, CT + 2 * j:CT + 2 * j + 2,
                                     512 * hf:512 * (hf + 1)],
                                start=False, stop=(j == 2),
                                perf_mode=DR,
                            )
                    vdrain(vT_c, ps[m], m)
                for m in range(3, CT):
                    pskv = psp.tile([128, CHUNK], F32, name="pskv", tag="pskv",
                                    bufs=BUFS_PSKV)
                    vproj_m(pskv, yq_c, m, [(0, 0), (CT, 0), (0, CT)])
                    vdrain(vT_c, pskv, m)
                return vT_c

            def fold_wps(g):
                # fold 1/S into the projection weights for head pair g
                S_tot = wp.tile([128, 1], F32, name="S_tot", tag="S_tot", bufs=2)
                nc.vector.tensor_reduce(S_tot[:], S_parts[g][:],
                                        axis=mybir.AxisListType.X,
                                        op=mybir.AluOpType.add)
                R_g = wp.tile([128, 1], F32, name="R_g", tag="R_g", bufs=2)
                nc.vector.reciprocal(R_g[:], S_tot[:])
                nc.vector.tensor_scalar_mul(wps_sb[:, g, :], wp_sb[:, g, :],
                                            R_g[:])
                nc.scalar.activation(wps8[:, g, :], wps_sb[:, g, :],
                                     mybir.ActivationFunctionType.Copy,
                                     bias=0.0, scale=1.0)
                nc.gpsimd.tensor_tensor(wps8[:, CT + g, :], wps_sb[:, g, :],
                                        wps8[:, g, :],
                                        op=mybir.AluOpType.subtract)

            def scores(ci, yq_c, vT_c):
                # half-chunk (512) score tiles in fp8 DoubleRow: 3 passes
                # contract the full K=768 against the hi yT split.
                # On the last chunk the U quantization is deferred into the
                # out-projection phase (its consumer is outproj n=3) so the
                # scores->outproj boundary only waits on the wps8 fold.
                t0, w = CHUNKS[ci]
                nhf = w // 512
                deferred = []
                for g in range(PAIRS):
                    pss2 = [psp.tile([128, 512], F32, name="pss", tag="pss",
                                     bufs=2 * BUFS_PSS) for _ in range(nhf)]
                    for j in range(3):
                        for hf in range(nhf):
                            nc.tensor.matmul(
                                pss2[hf][:],
                                A_sb[:, 2 * j:2 * j + 2, 128 * g:128 * (g + 1)],
                                yq_c[:, 2 * j:2 * j + 2,
                                     512 * hf:512 * (hf + 1)],
                                start=(j == 0), stop=(j == 2),
                                perf_mode=DR,
                            )
                    e_sb = wp.tile([128, w], BF16, name="e_sb", tag="e_sb",
                                   bufs=BUFS_E)
                    for hf in range(nhf):
                        slot = SLOT0[ci] + hf
                        nc.scalar.activation(e_sb[:, 512 * hf:512 * (hf + 1)],
                                             pss2[hf][:],
                                             mybir.ActivationFunctionType.Exp,
                                             bias=lbias[:], scale=EXPSCALE,
                                             accum_out=S_parts[g][:, slot:
                                                                  slot + 1])
                    tok = slice(t0, t0 + w)

                    def uquant(g=g, e_sb=e_sb, vT=vT_c[g], tok=tok, w=w):
                        P_t = wp.tile([128, w], BF16, name="pmul",
                                      tag="pmul", bufs=3)
                        nc.vector.tensor_mul(P_t[:], e_sb[:], vT[:])
                        nc.gpsimd.tensor_copy(U8[:, g, tok], P_t[:])
                        nc.vector.tensor_tensor(U8[:, CT + g, tok], P_t[:],
                                                U8[:, g, tok],
                                                op=mybir.AluOpType.subtract)

                    if ci == len(CHUNKS) - 1:
                        # g's row-sums are complete — fold 1/S + quantize the
                        # projection weights while the PE runs the remaining
                        # pairs; U quantization waits for the outproj phase.
                        fold_wps(g)
                        deferred.append(uquant)
                    else:
                        uquant()
                return deferred

            # chunk-0 v-projection first: its inputs lead the DMA stream, so
            # the PE starts as early as possible.
            vT_next = vproj0(0, yq_next)

            # ---- qT = (W_q @ xT) * scale ------------------------------------
            for m in range(CT):
                psq = psp.tile([128, N1], F32, name="psq", tag="pss", bufs=2 * BUFS_PSS)
                for kk in range(CT):
                    nc.tensor.matmul(
                        psq[:],
                        wq_sb[:, kk, 128 * m:128 * (m + 1)],
                        xT_sb[:, kk, :],
                        start=(kk == 0), stop=(kk == CT - 1),
                    )
                nc.scalar.activation(qbd[0:64, m, 0:64], psq[0:64, :],
                                     mybir.ActivationFunctionType.Copy,
                                     bias=0.0, scale=SCALE)
                nc.scalar.activation(qbd[64:128, m, 64:128], psq[64:128, :],
                                     mybir.ActivationFunctionType.Copy,
                                     bias=0.0, scale=SCALE)

            # ---- A_h = q_h @ W_k_h, quantized to e4m3(128*A) ----------------
            for kk in range(CT):
                psA = psp.tile([128, C], F32, name="psA", tag="pskv", bufs=BUFS_PSKV)
                for g in range(PAIRS):
                    nc.tensor.matmul(
                        psA[:, 128 * g:128 * (g + 1)],
                        wk_sb[:, g, 128 * kk:128 * (kk + 1)],
                        qbd[:, g, :],
                        start=True, stop=True,
                    )
                if kk % 2 == 0:
                    nc.scalar.activation(A_sb[:, kk, :], psA[:],
                                         mybir.ActivationFunctionType.Copy,
                                         bias=0.0, scale=SA)
                else:
                    nc.vector.tensor_scalar_mul(A_sb[:, kk, :], psA[:], SA)

            # ---- stream over token chunks -----------------------------------
            NC_S = len(CHUNKS)
            for ci in range(NC_S):
                yq_c, vT_c = yq_next, vT_next
                if ci + 1 < NC_S:
                    yq_next = chunk_dma(ci + 1)
                if ci == 1:
                    # proj weights: after y1 (tight), before y2 (slack).
                    nc.sync.dma_start(wp_sb[:], t6(wprojT_d[:, :]))
                    nc.sync.dma_start(bias_sb[:], t6(bproj_d[:, :]))
                deferred = scores(ci, yq_c, vT_c)
                if ci + 1 < NC_S:
                    vT_next = vproj(ci + 1, yq_next)

            # ---- outT = W_proj_scaled @ U + b -------------------------------
            # n outer so output stores batch per chunk; the last chunk stores
            # per m-tile to keep the kernel tail short.
            for n in range(NCH):
                tok = slice(CHUNK * n, CHUNK * (n + 1))
                last = (n == NCH - 1)
                outc = None
                for m in range(CT):
                    if m % 3 == 0 and not last:
                        # 3-m staging halves: finer slot rotation than a full
                        # [CT, CHUNK] tile, and each store is only 0.75 MB.
                        outc = wp.tile([128, 3, CHUNK], BF16, name="outc",
                                       tag="outc", bufs=3)
                    psq2 = psp.tile([128, CHUNK], F32, name="psq2", tag="pskv",
                                    bufs=BUFS_PSKV)
                    for ti, (lb, rb) in enumerate([(0, 0), (0, CT), (CT, 0)]):
                        for j in range(3):
                            for hf in range(2):
                                nc.tensor.matmul(
                                    psq2[:, 512 * hf:512 * (hf + 1)],
                                    wps8[:, lb + 2 * j:lb + 2 * j + 2,
                                         128 * m:128 * (m + 1)],
                                    U8[:, rb + 2 * j:rb + 2 * j + 2,
                                       CHUNK * n + 512 * hf:
                                       CHUNK * n + 512 * (hf + 1)],
                                    start=(ti == 0 and j == 0),
                                    stop=(ti == 2 and j == 2),
                                    perf_mode=DR,
                                )
                    if last:
                        # last chunk: per-m stores keep the kernel tail short;
                        # the final tile drains in halves on both engines and
                        # stores via both DMA queues.
                        outm = wp.tile([128, CHUNK], BF16, name="outm",
                                       tag="outm", bufs=3)
                        if m == CT - 1:
                            nc.scalar.activation(
                                outm[:, :512], psq2[:, :512],
                                mybir.ActivationFunctionType.Identity,
                                bias=bias_sb[:, m, :], scale=SD)
                            nc.vector.tensor_scalar(
                                outm[:, 512:], psq2[:, 512:], SD,
                                bias_sb[:, m, :], op0=mybir.AluOpType.mult,
                                op1=mybir.AluOpType.add)
                            nc.sync.dma_start(
                                outT_d[128 * m:128 * (m + 1),
                                       CHUNK * n:CHUNK * n + 512],
                                outm[:, :512])
                            nc.sync.dma_start(
                                outT_d[128 * m:128 * (m + 1),
                                       CHUNK * n + 512:CHUNK * (n + 1)],
                                outm[:, 512:])
                        else:
                            if m % 2 == 0:
                                nc.scalar.activation(
                                    outm[:], psq2[:],
                                    mybir.ActivationFunctionType.Identity,
                                    bias=bias_sb[:, m, :], scale=SD)
                            else:
                                nc.vector.tensor_scalar(
                                    outm[:], psq2[:], SD, bias_sb[:, m, :],
                                    op0=mybir.AluOpType.mult,
                                    op1=mybir.AluOpType.add)
                            nc.scalar.dma_start(
                                outT_d[128 * m:128 * (m + 1), tok], outm[:])
                    else:
                        if m % 2 == 0:
                            nc.scalar.activation(
                                outc[:, m % 3, :], psq2[:],
                                mybir.ActivationFunctionType.Identity,
                                bias=bias_sb[:, m, :], scale=SD)
                        else:
                            nc.vector.tensor_scalar(
                                outc[:, m % 3, :], psq2[:], SD,
                                bias_sb[:, m, :], op0=mybir.AluOpType.mult,
                                op1=mybir.AluOpType.add)
                        if m % 3 == 2:
                            h3 = m // 3
                            nc.scalar.dma_start(
                                outT_d[384 * h3:384 * (h3 + 1), tok].rearrange(
                                    "(t p) c -> p t c", p=128),
                                outc[:])
                if n == 0:
                    # chunk-3 U quantization runs during the outproj phase
                    for fn_ in deferred:
                        fn_()

    nc.compile()
    return nc


def kernel(x, y, W_qkv, W_proj, b_proj):
    if "nc" not in _CACHE:
        _CACHE["nc"] = _build()
    nc = _CACHE["nc"]
    in_maps = make_in_maps(x, y, W_qkv, W_proj, b_proj)
    # The axon-tunneled devices occasionally fail one execution with a
    # transient NRT_EXEC_UNIT_UNRECOVERABLE; a clean retry succeeds.
    last_err = None
    for attempt in range(3):
        try:
            res = run_bass_kernel_spmd(nc, in_maps, core_ids=list(range(B)))
            break
        except Exception as e:  # noqa: BLE001
            last_err = e
            import time
            time.sleep(2.0 * (attempt + 1))
    else:
        raise last_err
    out = np.empty((B, N2, C), np.float32)
    for i in range(B):
        out[i] = res.results[i]["outT"].astype(np.float32).T
    return out


def make_in_maps(x, y, W_qkv, W_proj, b_proj):
    bf = ml_dtypes.bfloat16
    f8 = ml_dtypes.float8_e4m3

    def q8(a):
        hi = a.astype(f8)
        lo = (a - hi.astype(np.float32)).astype(f8)
        return hi, lo

    W_qkv = np.asarray(W_qkv, np.float32)
    wq = np.ascontiguousarray(W_qkv[:C].T).astype(bf)
    wk = np.ascontiguousarray(W_qkv[C:2 * C]).astype(bf)
    wv_hi, wv_lo = q8(np.ascontiguousarray(W_qkv[2 * C:].T) * SWV)
    wvq = np.concatenate([wv_hi, wv_lo], axis=0)
    wprojT = np.ascontiguousarray(
        np.asarray(W_proj, np.float32).T * SWP).astype(bf)
    bproj = np.asarray(b_proj, np.float32).reshape(C, 1)

    in_maps = []
    for i in range(B):
        xT = np.ascontiguousarray(np.asarray(x[i], np.float32).T).astype(bf)
        yT = np.ascontiguousarray(np.asarray(y[i], np.float32).T) * SY
        y_hi, y_lo = q8(yT)
        yq = np.concatenate([y_hi, y_lo], axis=0)
        in_maps.append({
            "xT": xT,
            "yq": yq,
            "wq": wq,
            "wk": wk,
            "wvq": wvq,
            "wprojT": wprojT,
            "bproj": bproj,
        })
    return in_maps


# revision 24
# speedup vs baseline: 1.1008x; 1.1008x over previous
"""Trainium2 Bass kernel for nn_CrossAttention (B=8, N1=64, N2=4096, C=768, H=12).

Strategy: data-parallel over batch across 8 NeuronCores (one item per core,
no collectives). All activations kept transposed (channels on partitions,
tokens on the free dim) so every matmul contracts over SBUF partitions.

Key algebraic restructurings (exploiting that the module's combine with v is
ELEMENTWISE, attn_t * v, not attn @ v):

  1. scores_h = q_h @ k_h^T = (q_h @ W_k_h) @ yT = A_h @ yT.  A = qT^T @ W_k
     is a tiny [768,768] precompute; scores then contract over the full
     K=128 partitions with the SAME moving operand (yT chunks) as the
     v-projection — k is never materialized.
  2. softmax normalization is deferred: U_h = exp(s_h) * vT_h is accumulated
     unnormalized; row-sums S come free via ACT's fused accum_out; 1/S is
     folded into the projection weights (O(C^2), not O(C*N2)).

fp8 acceleration (DoubleRow perf mode, 2 contraction tiles per pass):
  - v-projection runs as an error-compensated fp8 "double-double":
    host ships hi/lo e4m3 splits of both W_v (x64) and yT (x16), and the
    kernel accumulates Whi.yhi + Wlo.yhi + Whi.ylo (9 DoubleRow passes vs
    12 bf16 passes worth of PE rows) -- quantization error ~eps^2.
  - scores run in plain fp8: A is quantized on device (x128) against the
    hi yT split. Score errors are absolute-small and further damped by
    softmax's exp; end-to-end rel err stays ~1.2e-2 (gate 2e-2).
  - the output projection stays bf16.

Matmuls accumulate in f32 PSUM; softmax statistics in f32.
DMAs are batched; the input stream is split across the SP and ACT queues so
the first PE work (qT projection) and the chunk-0 v-projection hi terms can
start as early as possible.
"""

import numpy as np
import ml_dtypes

import concourse.bass as bass
import concourse.mybir as mybir
import concourse.tile as tile
from concourse import bacc
from concourse.bass_utils import run_bass_kernel_spmd

BF16 = mybir.dt.bfloat16
FP8 = mybir.dt.float8e4
F32 = mybir.dt.float32
DR = mybir.MatmulPerfMode.DoubleRow

B, N1, N2, C, H = 8, 64, 4096, 768, 12
HD = C // H              # 64
SCALE = HD ** -0.5       # 1/8
CT = C // 128            # 6 partition tiles of channels
CT2 = 2 * CT             # 12 fp8 k-tiles (hi | lo)
CHUNK = 1024             # tokens per outproj block
NCH = N2 // CHUNK        # 4 outproj blocks
# scores/vproj stream chunks: last chunk halved so the scores->outproj
# boundary only waits for 6 exps instead of 12
CHUNKS = [(0, 1024), (1024, 1024), (2048, 1024), (3072, 1024)]
SLOT0 = [0, 2, 4, 6]     # S_parts slot base per chunk (one per 512 tokens)
PAIRS = CT               # 6 head pairs (2 heads per 128-partition tile)

# quantization scales (powers of two; folded back out exactly)
SY = 16.0                # yT shipped as e4m3(16*y) + residual
SWV = 64.0               # W_v^T shipped as e4m3(64*Wv) + residual
SA = 128.0               # A quantized on device as e4m3(128*A)
VDRAIN = 1.0 / (SY * SWV)        # 2^-10: v psum -> true v
# qbd already folds SCALE=1/8, so psum_s = (128*A)@(16*y) = 2048*s
EXPSCALE = 1.0 / (SA * SY)       # 2^-11
ALPHA = 8.0                      # E' = 8*E via exp bias ln(8): fp8 range for U
SWP = 2.0 ** 18                  # W_proj^T shipped pre-scaled by 2^18
SD = 1.0 / SWP                   # out-proj psum drain scale

BUFS_YT = 2
BUFS_VT = 2
BUFS_E = 6
BUFS_PSKV = 3
BUFS_PSS = 1

_CACHE = {}


def _build():
    nc = bacc.Bacc("TRN2", target_bir_lowering=False, debug=False)

    xT_d = nc.dram_tensor("xT", [C, N1], BF16, kind="ExternalInput")
    # yq: rows 0..C-1 = e4m3(16*yT), rows C..2C-1 = residual
    yq_d = nc.dram_tensor("yq", [2 * C, N2], FP8, kind="ExternalInput")
    # wq: W_q^T layout [c_in, c_out]
    wq_d = nc.dram_tensor("wq", [C, C], BF16, kind="ExternalInput")
    # wk: natural layout [c_out, c_in] = W_qkv[C:2C, :]
    wk_d = nc.dram_tensor("wk", [C, C], BF16, kind="ExternalInput")
    # wvq: [Wv_hi | Wv_lo] stacked on rows; each [c_in, c_out] = e4m3(64*Wv^T)
    wvq_d = nc.dram_tensor("wvq", [2 * C, C], FP8, kind="ExternalInput")
    wprojT_d = nc.dram_tensor("wprojT", [C, C], BF16, kind="ExternalInput")
    bproj_d = nc.dram_tensor("bproj", [C, 1], F32, kind="ExternalInput")
    outT_d = nc.dram_tensor("outT", [C, N2], BF16, kind="ExternalOutput")

    def t6(ap):  # [768, X] dram view -> [128, 6, X] partition-tiled view
        return ap.rearrange("(t p) c -> p t c", p=128)

    with tile.TileContext(nc) as tc:
        with (
            tc.tile_pool(name="persist", bufs=1) as pp,
            tc.tile_pool(name="work", bufs=2) as wp,
            tc.tile_pool(name="psum", bufs=2, space=bass.MemorySpace.PSUM) as psp,
        ):
            # ---- persistent tiles (partition-tiled: [:, kk, :] = rows of 128)
            wq_sb = pp.tile([128, CT, C], BF16, name="wq", tag="wq")
            wv_sb = pp.tile([128, CT2, C], FP8, name="wv", tag="wv")
            wk_sb = pp.tile([128, CT, C], BF16, name="wk", tag="wk")
            wp_sb = pp.tile([128, CT, C], BF16, name="wpr", tag="wpr")
            wps_sb = pp.tile([128, CT, C], BF16, name="wps", tag="wps")
            A_sb = pp.tile([128, CT, C], FP8, name="A", tag="A")
            xT_sb = pp.tile([128, CT, N1], BF16, name="xTs", tag="xTs")
            bias_sb = pp.tile([128, CT, 1], F32, name="biass", tag="biass")
            # block-diagonal q: qbd[0:64, g, 0:64] = qT head 2g,
            # qbd[64:128, g, 64:128] = qT head 2g+1, zeros elsewhere.
            # Lets A-prep contract K=128 in one clean full-array matmul.
            qbd = pp.tile([128, CT, 128], BF16, name="qbd", tag="qbd")
            U8 = pp.tile([128, CT2, N2], FP8, name="U8", tag="U8")
            wps8 = pp.tile([128, CT2, C], FP8, name="wps8", tag="wps8")
            S_parts = [pp.tile([128, 2 * NCH], F32, name=f"Sp{g}", tag=f"Sp{g}")
                       for g in range(PAIRS)]
            zbias = pp.tile([128, 1], F32, name="zbias", tag="zbias")
            lbias = pp.tile([128, 1], F32, name="lbias", tag="lbias")
            nc.gpsimd.memset(zbias[:], 0.0)
            nc.gpsimd.memset(lbias[:], float(np.log(ALPHA)))
            nc.gpsimd.memset(qbd[:], 0.0)

            # ---- batched weight/input DMAs ----------------------------------
            # ALL input transfers ride the sync (SP) queue: transfers
            # serialize on the shared DMA engines, so a single queue pins the
            # global order to the PE program's consumption order. Output
            # stores use the scalar queue (disjoint in time).
            nc.sync.dma_start(wv_sb[:, :CT, :], t6(wvq_d[:C, :]))

            def chunk_dma(ci):
                t0, w = CHUNKS[ci]
                yq_c = wp.tile([128, CT2, w], FP8, name="yqc", tag="yqc",
                               bufs=BUFS_YT)
                nc.sync.dma_start(
                    yq_c[:],
                    yq_d[:, t0:t0 + w].rearrange("(t p) c -> p t c", p=128))
                return yq_c

            # chunk 0 in hi / wv_lo / lo arrival order = vproj0 pass order.
            yq_next = wp.tile([128, CT2, CHUNK], FP8, name="yqc", tag="yqc",
                              bufs=BUFS_YT)
            nc.sync.dma_start(yq_next[:, :CT, :], t6(yq_d[:C, :CHUNK]))
            nc.sync.dma_start(wv_sb[:, CT:, :], t6(wvq_d[C:, :]))
            nc.sync.dma_start(yq_next[:, CT:, :], t6(yq_d[C:, :CHUNK]))

            nc.sync.dma_start(xT_sb[:], t6(xT_d[:, :]))
            nc.sync.dma_start(wq_sb[:], t6(wq_d[:, :]))
            nc.sync.dma_start(wk_sb[:], t6(wk_d[:, :]))

            def vproj_m(pskv, yq_c, m, terms, nhf=2):
                """Accumulate DoubleRow passes for out-tile m into pskv.

                terms: list of (lhs_base, rhs_base) k-tile base offsets:
                  (0, 0) = Whi.yhi, (CT, 0) = Wlo.yhi, (0, CT) = Whi.ylo.
                start on the first pass of the first term; stop on the last.
                """
                npass = 0
                total = 3 * len(terms)
                for (lb, rb) in terms:
                    for j in range(3):
                        for hf in range(nhf):
                            nc.tensor.matmul(
                                pskv[:, 512 * hf:512 * (hf + 1)],
                                wv_sb[:, lb + 2 * j:lb + 2 * j + 2,
                                      128 * m:128 * (m + 1)],
                                yq_c[:, rb + 2 * j:rb + 2 * j + 2,
                                     512 * hf:512 * (hf + 1)],
                                start=(npass == 0), stop=(npass == total - 1),
                                perf_mode=DR,
                            )
                        npass += 1

            def vdrain(vT_c, pskv, m, dve_only=False):
                if not dve_only and m % 2 == 0:
                    nc.scalar.activation(vT_c[m][:], pskv[:],
                                         mybir.ActivationFunctionType.Copy,
                                         bias=0.0, scale=VDRAIN)
                else:
                    nc.vector.tensor_scalar_mul(vT_c[m][:], pskv[:], VDRAIN)

            def vproj(ci, yq_c):
                w = CHUNKS[ci][1]
                vT_c = [wp.tile([128, w], BF16, name=f"vTc{m}", tag=f"vTc{m}",
                                bufs=BUFS_VT) for m in range(CT)]
                for m in range(CT):
                    pskv = psp.tile([128, w], F32, name="pskv", tag="pskv",
                                    bufs=BUFS_PSKV)
                    vproj_m(pskv, yq_c, m,
                            [(0, 0), (CT, 0), (0, CT)], nhf=w // 512)
                    vdrain(vT_c, pskv, m)
                return vT_c

            def vproj0(c, yq_c):
                """Chunk-0 variant ordered around DMA arrival: hi terms for
                m0-m2 run before y_lo lands; their lo-y passes close after."""
                vT_c = [wp.tile([128, CHUNK], BF16, name=f"vTc{m}", tag=f"vTc{m}",
                                bufs=BUFS_VT) for m in range(CT)]
                ps = [psp.tile([128, CHUNK], F32, name="pskv", tag="pskv",
                               bufs=BUFS_PSKV) for _ in range(3)]
                for (lb, rb) in [(0, 0), (CT, 0)]:
                    for m in range(3):
                        for j in range(3):
                            for hf in range(2):
                                nc.tensor.matmul(
                                    ps[m][:, 512 * hf:512 * (hf + 1)],
                                    wv_sb[:, lb + 2 * j:lb + 2 * j + 2,
                                          128 * m:128 * (m + 1)],
                                    yq_c[:, rb + 2 * j:rb + 2 * j + 2,
                                         512 * hf:512 * (hf + 1)],
                                    start=(lb == 0 and j == 0), stop=False,
                                    perf_mode=DR,
                                )
                for m in range(3):
                    for j in range(3):
                        for hf in range(2):
                            nc.tensor.matmul(
                                ps[m][:, 512 * hf:512 * (hf + 1)],
                                wv_sb[:, 2 * j:2 * j + 2, 128 * m:128 * (m + 1)],
                                yq_c[:, CT + 2 * j:CT + 2 * j + 2,
                                     512 * hf:512 * (hf + 1)],
                                start=False, stop=(j == 2),
                                perf_mode=DR,
                            )
                    vdrain(vT_c, ps[m], m)
                for m in range(3, CT):
                    pskv = psp.tile([128, CHUNK], F32, name="pskv", tag="pskv",
                                    bufs=BUFS_PSKV)
                    vproj_m(pskv, yq_c, m, [(0, 0), (CT, 0), (0, CT)])
                    vdrain(vT_c, pskv, m)
                return vT_c

            def fold_wps(g):
                # fold 1/S into the projection weights for head pair g
                S_tot = wp.tile([128, 1], F32, name="S_tot", tag="S_tot", bufs=2)
                nc.vector.tensor_reduce(S_tot[:], S_parts[g][:],
                                        axis=mybir.AxisListType.X,
                                        op=mybir.AluOpType.add)
                R_g = wp.tile([128, 1], F32, name="R_g", tag="R_g", bufs=2)
                nc.vector.reciprocal(R_g[:], S_tot[:])
                nc.vector.tensor_scalar_mul(wps_sb[:, g, :], wp_sb[:, g, :],
                                            R_g[:])
                nc.scalar.activation(wps8[:, g, :], wps_sb[:, g, :],
                                     mybir.ActivationFunctionType.Copy,
                                     bias=0.0, scale=1.0)
                nc.gpsimd.tensor_tensor(wps8[:, CT + g, :], wps_sb[:, g, :],
                                        wps8[:, g, :],
                                        op=mybir.AluOpType.subtract)

            def scores(ci, yq_c, vT_c):
                # half-chunk (512) score tiles in fp8 DoubleRow: 3 passes
                # contract the full K=768 against the hi yT split.
                # On the last chunk the U quantization is deferred into the
                # out-projection phase (its consumer is outproj n=3) so the
                # scores->outproj boundary only waits on the wps8 fold.
                t0, w = CHUNKS[ci]
                nhf = w // 512
                deferred = []
                for g in range(PAIRS):
                    pss2 = [psp.tile([128, 512], F32, name="pss", tag="pss",
                                     bufs=2 * BUFS_PSS) for _ in range(nhf)]
                    for j in range(3):
                        for hf in range(nhf):
                            nc.tensor.matmul(
                                pss2[hf][:],
                                A_sb[:, 2 * j:2 * j + 2, 128 * g:128 * (g + 1)],
                                yq_c[:, 2 * j:2 * j + 2,
                                     512 * hf:512 * (hf + 1)],
                                start=(j == 0), stop=(j == 2),
                                perf_mode=DR,
                            )
                    e_sb = wp.tile([128, w], BF16, name="e_sb", tag="e_sb",
                                   bufs=BUFS_E)
                    for hf in range(nhf):
                        slot = SLOT0[ci] + hf
                        nc.scalar.activation(e_sb[:, 512 * hf:512 * (hf + 1)],
                                             pss2[hf][:],
                                             mybir.ActivationFunctionType.Exp,
                                             bias=lbias[:], scale=EXPSCALE,
                                             accum_out=S_parts[g][:, slot:
                                                                  slot + 1])
                    tok = slice(t0, t0 + w)

                    def uquant(vTl, g=g, e_sb=e_sb, tok=tok, w=w):
                        P_t = wp.tile([128, w], BF16, name="pmul",
                                      tag="pmul", bufs=3)
                        nc.vector.tensor_mul(P_t[:], e_sb[:], vTl[g][:])
                        nc.gpsimd.tensor_copy(U8[:, g, tok], P_t[:])
                        nc.vector.tensor_tensor(U8[:, CT + g, tok], P_t[:],
                                                U8[:, g, tok],
                                                op=mybir.AluOpType.subtract)

                    if ci == len(CHUNKS) - 1:
                        # g's row-sums are complete — fold 1/S + quantize the
                        # projection weights while the PE runs the remaining
                        # pairs; U quantization waits for the outproj phase.
                        fold_wps(g)
                        deferred.append(uquant)
                    else:
                        uquant(vT_c)
                return deferred

            # chunk-0 v-projection first: its inputs lead the DMA stream, so
            # the PE starts as early as possible.
            vT_next = vproj0(0, yq_next)

            # ---- qT = (W_q @ xT) * scale ------------------------------------
            for m in range(CT):
                psq = psp.tile([128, N1], F32, name="psq", tag="pss", bufs=2 * BUFS_PSS)
                for kk in range(CT):
                    nc.tensor.matmul(
                        psq[:],
                        wq_sb[:, kk, 128 * m:128 * (m + 1)],
                        xT_sb[:, kk, :],
                        start=(kk == 0), stop=(kk == CT - 1),
                    )
                nc.scalar.activation(qbd[0:64, m, 0:64], psq[0:64, :],
                                     mybir.ActivationFunctionType.Copy,
                                     bias=0.0, scale=SCALE)
                nc.scalar.activation(qbd[64:128, m, 64:128], psq[64:128, :],
                                     mybir.ActivationFunctionType.Copy,
                                     bias=0.0, scale=SCALE)

            # ---- A_h = q_h @ W_k_h, quantized to e4m3(128*A) ----------------
            for kk in range(CT):
                psA = psp.tile([128, C], F32, name="psA", tag="pskv", bufs=BUFS_PSKV)
                for g in range(PAIRS):
                    nc.tensor.matmul(
                        psA[:, 128 * g:128 * (g + 1)],
                        wk_sb[:, g, 128 * kk:128 * (kk + 1)],
                        qbd[:, g, :],
                        start=True, stop=True,
                    )
                if kk % 2 == 0:
                    nc.scalar.activation(A_sb[:, kk, :], psA[:],
                                         mybir.ActivationFunctionType.Copy,
                                         bias=0.0, scale=SA)
                else:
                    nc.vector.tensor_scalar_mul(A_sb[:, kk, :], psA[:], SA)

            # ---- stream over token chunks -----------------------------------
            # The last chunk's scores (matmuls + exps + 1/S folds) run BEFORE
            # its v-projection: U quantization is deferred, so the 12 exps
            # and the wps8 fold overlap vproj(3)'s PE work and the
            # out-projection starts with no boundary stall.
            NC_S = len(CHUNKS)
            for ci in range(NC_S - 1):
                yq_c, vT_c = yq_next, vT_next
                yq_next = chunk_dma(ci + 1)
                if ci == 1:
                    # proj weights: after y1 (tight), before y2 (slack).
                    nc.sync.dma_start(wp_sb[:], t6(wprojT_d[:, :]))
                    nc.sync.dma_start(bias_sb[:], t6(bproj_d[:, :]))
                scores(ci, yq_c, vT_c)
                if ci + 1 < NC_S - 1:
                    vT_next = vproj(ci + 1, yq_next)
            deferred = scores(NC_S - 1, yq_next, None)
            vT_last = vproj(NC_S - 1, yq_next)

            # ---- outT = W_proj_scaled @ U + b -------------------------------
            # n outer so output stores batch per chunk; the last chunk stores
            # per m-tile to keep the kernel tail short.
            for n in range(NCH):
                tok = slice(CHUNK * n, CHUNK * (n + 1))
                last = (n == NCH - 1)
                outc = None
                for m in range(CT):
                    if m % 3 == 0 and not last:
                        # 3-m staging halves: finer slot rotation than a full
                        # [CT, CHUNK] tile, and each store is only 0.75 MB.
                        outc = wp.tile([128, 3, CHUNK], BF16, name="outc",
                                       tag="outc", bufs=3)
                    psq2 = psp.tile([128, CHUNK], F32, name="psq2", tag="pskv",
                                    bufs=BUFS_PSKV)
                    for ti, (lb, rb) in enumerate([(0, 0), (0, CT), (CT, 0)]):
                        for j in range(3):
                            for hf in range(2):
                                nc.tensor.matmul(
                                    psq2[:, 512 * hf:512 * (hf + 1)],
                                    wps8[:, lb + 2 * j:lb + 2 * j + 2,
                                         128 * m:128 * (m + 1)],
                                    U8[:, rb + 2 * j:rb + 2 * j + 2,
                                       CHUNK * n + 512 * hf:
                                       CHUNK * n + 512 * (hf + 1)],
                                    start=(ti == 0 and j == 0),
                                    stop=(ti == 2 and j == 2),
                                    perf_mode=DR,
                                )
                    if last:
                        # last chunk: per-m stores keep the kernel tail short;
                        # the final tile drains in halves on both engines and
                        # stores via both DMA queues.
                        outm = wp.tile([128, CHUNK], BF16, name="outm",
                                       tag="outm", bufs=3)
                        if m == CT - 1:
                            nc.scalar.activation(
                                outm[:, :512], psq2[:, :512],
                                mybir.ActivationFunctionType.Identity,
                                bias=bias_sb[:, m, :], scale=SD)
                            nc.vector.tensor_scalar(
                                outm[:, 512:], psq2[:, 512:], SD,
                                bias_sb[:, m, :], op0=mybir.AluOpType.mult,
                                op1=mybir.AluOpType.add)
                            nc.sync.dma_start(
                                outT_d[128 * m:128 * (m + 1),
                                       CHUNK * n:CHUNK * n + 512],
                                outm[:, :512])
                            nc.sync.dma_start(
                                outT_d[128 * m:128 * (m + 1),
                                       CHUNK * n + 512:CHUNK * (n + 1)],
                                outm[:, 512:])
                        else:
                            if m % 2 == 0:
                                nc.scalar.activation(
                                    outm[:], psq2[:],
                                    mybir.ActivationFunctionType.Identity,
                                    bias=bias_sb[:, m, :], scale=SD)
                            else:
                                nc.vector.tensor_scalar(
                                    outm[:], psq2[:], SD, bias_sb[:, m, :],
                                    op0=mybir.AluOpType.mult,
                                    op1=mybir.AluOpType.add)
                            nc.scalar.dma_start(
                                outT_d[128 * m:128 * (m + 1), tok], outm[:])
                    else:
                        if m % 2 == 0:
                            nc.scalar.activation(
                                outc[:, m % 3, :], psq2[:],
                                mybir.ActivationFunctionType.Identity,
                                bias=bias_sb[:, m, :], scale=SD)
                        else:
                            nc.vector.tensor_scalar(
                                outc[:, m % 3, :], psq2[:], SD,
                                bias_sb[:, m, :], op0=mybir.AluOpType.mult,
                                op1=mybir.AluOpType.add)
                        if m % 3 == 2:
                            h3 = m // 3
                            nc.scalar.dma_start(
                                outT_d[384 * h3:384 * (h3 + 1), tok].rearrange(
                                    "(t p) c -> p t c", p=128),
                                outc[:])
                if n == 0:
                    # chunk-3 U quantization runs during the outproj phase
                    for fn_ in deferred:
                        fn_(vT_last)

    nc.compile()
    return nc


def kernel(x, y, W_qkv, W_proj, b_proj):
    if "nc" not in _CACHE:
        _CACHE["nc"] = _build()
    nc = _CACHE["nc"]
    in_maps = make_in_maps(x, y, W_qkv, W_proj, b_proj)
    # The axon-tunneled devices occasionally fail one execution with a
    # transient NRT_EXEC_UNIT_UNRECOVERABLE; a clean retry succeeds.
    last_err = None
    for attempt in range(3):
        try:
            res = run_bass_kernel_spmd(nc, in_maps, core_ids=list(range(B)))
            break
        except Exception as e:  # noqa: BLE001
            last_err = e
            import time
            time.sleep(2.0 * (attempt + 1))
    else:
        raise last_err
    out = np.empty((B, N2, C), np.float32)
    for i in range(B):
        out[i] = res.results[i]["outT"].astype(np.float32).T
    return out


def make_in_maps(x, y, W_qkv, W_proj, b_proj):
    bf = ml_dtypes.bfloat16
    f8 = ml_dtypes.float8_e4m3

    def q8(a):
        hi = a.astype(f8)
        lo = (a - hi.astype(np.float32)).astype(f8)
        return hi, lo

    W_qkv = np.asarray(W_qkv, np.float32)
    wq = np.ascontiguousarray(W_qkv[:C].T).astype(bf)
    wk = np.ascontiguousarray(W_qkv[C:2 * C]).astype(bf)
    wv_hi, wv_lo = q8(np.ascontiguousarray(W_qkv[2 * C:].T) * SWV)
    wvq = np.concatenate([wv_hi, wv_lo], axis=0)
    wprojT = np.ascontiguousarray(
        np.asarray(W_proj, np.float32).T * SWP).astype(bf)
    bproj = np.asarray(b_proj, np.float32).reshape(C, 1)

    in_maps = []
    for i in range(B):
        xT = np.ascontiguousarray(np.asarray(x[i], np.float32).T).astype(bf)
        yT = np.ascontiguousarray(np.asarray(y[i], np.float32).T) * SY
        y_hi, y_lo = q8(yT)
        yq = np.concatenate([y_hi, y_lo], axis=0)
        in_maps.append({
            "xT": xT,
            "yq": yq,
            "wq": wq,
            "wk": wk,
            "wvq": wvq,
            "wprojT": wprojT,
            "bproj": bproj,
        })
    return in_maps
